# revision 16
# baseline (speedup 1.0000x reference)
"""AffinityCosineLoss on 8 Trainium2 NeuronCores — fp8 streaming matmul.

Math: with zn = l2norm(y_pred[:, :192]), latent = (zn@zn.T + 1)/2,
target[i,j] = 0.2 (both bg) / 0.01 (one bg) / lookup[y_i,y_j] (both valid),
loss = sum_{i<j} |latent - target| / (B*(B-1)/2).

The entire pairwise map latent - target is a single K=323 contraction
P.T @ Q, fully packed on the HOST (fp32 math, then fp8 cast):
  rows   0:192  P = zn_i.T            Q = 0.5 * zn_j.T
  row    192    P = 1                 Q = 0.5            (the +1/2 of latent)
  row    193    P = b_i               Q = -0.01 - 0.18*b_j
  row    194    P = 1                 Q = -0.01*b_j      (b = is_background)
  rows 195:323  P = onehot(y_i)       Q = -lookup[:, y_j] * valid_j
The asymmetric 1.0 x 0.5 const split keeps the fp8 constants exact.
K chunks: A = rows 0:128, B = rows 128:256, C = rows 256:323 (67).
Plain matmuls (no DoubleRow): full 128-col stationaries trigger the
compiler's Fast Weight Load, and --enable-ldw-opt dedupes the repeated
stationary across the slot-pair inner loop.

Sharding (triangle/cyclic): the 4096x4096 pair matrix is an 8x8 grid of
512x512 super-blocks. Core r computes blocks (r, (r+d) mod 8) for d=0..4;
the d=4 slot is zero-padded on cores 4..7. The x2 weight of off-diagonal
slots is baked into the Q columns (|2x| = 2|x|), so the device just
abs-sums everything. Host: total = sum - diag_correction, /2, /npairs.

Device: 20 out-tiles [128,512] in 3 waves of slots {0,1},{2,3},{4};
PSUM units [128,2,512] rotate through all 8 banks (tag bufs=4, first
rotation slot doubles as the PE-warmup target). Drains (abs + sum into
one acc column) alternate ACT (activation Abs accum_out) / DVE
(tensor_reduce) and overlap the next wave's matmuls.
"""

import functools

import ml_dtypes
import numpy as np

B = 4096
D = 256
L = 128
D_USE = 192  # int(D * 0.75)
NB = 8  # super-block grid (512 rows each)
BLK = B // NB  # 512
NSLOT = 5  # col slots per core (d = 0..4)
NCOL = NSLOT * BLK  # 2560
N_CORES = 8
NORM_EPS = 1e-8

KT = 323  # contraction rows
KC = 67  # rows of chunk C
NUNIT = 2 * NSLOT  # drain units: (slot, strip-half)

FP8 = ml_dtypes.float8_e4m3


def _enable_ldw_opt():
    """Flip walrus --enable-ldw-opt to true (dedupes back-to-back LDWEIGHTS
    with identical stationary operands; the main loop is ordered for it)."""
    import concourse.bass_utils as bu

    if getattr(bu, "_ldw_opt_patched", False):
        return
    orig = bu.run_command

    def run_command_ldw(argv, **kwargs):
        argv = [
            a.replace("--enable-ldw-opt=false", "--enable-ldw-opt=true")
            if isinstance(a, str)
            else a
            for a in argv
        ]
        return orig(argv, **kwargs)

    bu.run_command = run_command_ldw
    bu._ldw_opt_patched = True


def _build_bass():
    import concourse.bacc as bacc
    import concourse.mybir as mybir
    import concourse.tile as tile

    # NOTE: walrus --enable-ldw-opt rejects these fp8 InstLdweights
    # ("not compatible with LDW optimization"), so it stays off; the
    # PE's 64-deep reorder window still pulls LDWEIGHTS ahead.

    fp32 = mybir.dt.float32
    bf16 = mybir.dt.bfloat16
    f8 = mybir.dt.float8e4

    nc = bacc.Bacc("TRN2", debug=False, num_devices=N_CORES)

    # all 128-partition input in one tensor, all 67-partition input in the
    # other: 2 + 2 wave-sliced DMA issues move everything
    i128_d = nc.dram_tensor(
        "i128", [128, 1024 + NSLOT * 1024], f8, kind="ExternalInput"
    )
    i67_d = nc.dram_tensor("i67", [KC, BLK + NSLOT * BLK], f8, kind="ExternalInput")
    acc_d = nc.dram_tensor("acc", [128, NUNIT], fp32, kind="ExternalOutput")

    AX = mybir.AxisListType
    ALU = mybir.AluOpType
    ACTF = mybir.ActivationFunctionType

    with tile.TileContext(nc) as tc:
        with (
            tc.tile_pool(name="cst", bufs=1) as cst,
            tc.tile_pool(name="work", bufs=1) as work,
            tc.tile_pool(name="ps", bufs=1, space="PSUM") as pps,
        ):
            # ---- SBUF tiles: one tile per DMA so dependency tracking and
            # write-hazard windows are exact (a matmul only waits on — and
            # only aliases — its own wave's buffer) ----
            pabs = work.tile([128, 2, BLK], f8)
            pcs = work.tile([KC, BLK], f8)
            qabw = [
                work.tile([128, 2 if w < 2 else 1, 2, BLK], f8, name=f"qab{w}")
                for w in range(3)
            ]
            qcw = [
                work.tile([KC, 2 if w < 2 else 1, BLK], f8, name=f"qc{w}")
                for w in range(3)
            ]
            acc = work.tile([128, NUNIT], fp32)

            # ---- engine warmup + input DMAs ----
            # All input rides the fast sync HWDGE ring, need-ordered.
            # Scalar keeps only the warmup activation (Abs table) + drains.
            wz = cst.tile([128, 512], f8)
            nc.gpsimd.memset(wz[:], 0.0)
            wact = cst.tile([128, 1], fp32)
            nc.gpsimd.memset(wact[:], 1.0)

            def dma_ab(w):
                ab = slice(1024 + w * 2048, 1024 + min((w + 1) * 2048, 5120))
                nc.sync.dma_start(qabw[w][:], i128_d.ap()[:, ab])

            def dma_c(w):
                c = slice(BLK + w * 1024, BLK + min((w + 1) * 1024, NSLOT * BLK))
                nc.sync.dma_start(qcw[w][:], i67_d.ap()[:, c])

            nc.sync.dma_start(pabs[:], i128_d.ap()[:, 0:1024])
            dma_ab(0)
            nc.sync.dma_start(pcs[:], i67_d.ap()[:, 0:BLK])
            dma_c(0)
            dma_ab(1)
            dma_c(1)
            dma_ab(2)
            dma_c(2)

            wabs = cst.tile([128, 1], fp32)
            nc.scalar.activation(wabs[:], wact[:], ACTF.Abs)

            wp = pps.tile([128, 2, BLK], fp32, tag="mm", bufs=4, name="wp")
            for wi in range(4):
                nc.tensor.matmul(
                    wp[:, wi % 2, :], wz[:, 0:128], wz[:], start=True, stop=True
                )

            # ---- main: 3 waves of slots {0,1}, {2,3}, {4} ----
            pending = []

            def drain(unit, u):
                if u % 2 == 0:
                    scr = work.tile([128, 2, BLK], bf16, tag="scr", bufs=2)
                    nc.scalar.activation(
                        scr[:], unit[:], ACTF.Abs, accum_out=acc[:, u : u + 1]
                    )
                else:
                    nc.vector.tensor_reduce(
                        acc[:, u : u + 1],
                        unit[:],
                        axis=AX.XY,
                        op=ALU.add,
                        apply_absolute_value=True,
                    )

            for wave in ((0, 1), (2, 3), (4,)):
                units = {}
                for g in wave:
                    for h in range(2):
                        units[(g, h)] = pps.tile(
                            [128, 2, BLK], fp32, tag="mm", bufs=4, name=f"u{g}_{h}"
                        )
                # drains of the previous wave overlap this wave's matmuls
                for unit, u in pending:
                    drain(unit, u)
                pending.clear()
                for m in range(4):
                    ms = slice(m * 128, (m + 1) * 128)
                    for c in range(3):
                        for g in wave:
                            w, gi = g // 2, g % 2
                            lhsT = pcs[:, ms] if c == 2 else pabs[:, c, ms]
                            rhs = (
                                qcw[w][:, gi, :]
                                if c == 2
                                else qabw[w][:, gi, c, :]
                            )
                            nc.tensor.matmul(
                                units[(g, m // 2)][:, m % 2, :],
                                lhsT,
                                rhs,
                                start=(c == 0),
                                stop=(c == 2),
                            )
                for g in wave:
                    for h in range(2):
                        pending.append((units[(g, h)], g * 2 + h))

            # ship the first 8 acc columns while wave 2 drains
            nc.sync.dma_start(acc_d.ap()[:, 0:8], acc[:, 0:8])
            for unit, u in pending:
                drain(unit, u)
            nc.sync.dma_start(acc_d.ap()[:, 8:NUNIT], acc[:, 8:NUNIT])

    nc.compile()
    return nc


@functools.lru_cache(maxsize=1)
def _get_nc():
    return _build_bass()


def _pack_pq(y_true, y_pred, lookup):
    """Global [KT, B] P and Q fp32 matrices (see module docstring)."""
    yt = np.asarray(y_true).astype(np.int64)
    yp = np.asarray(y_pred).astype(np.float32)[:, :D_USE]
    lk = np.asarray(lookup).astype(np.float32)

    n = np.maximum(np.sqrt((yp * yp).sum(axis=1, keepdims=True)), NORM_EPS)
    zn = (yp / n).T  # [192, B]
    bg = (yt == -1).astype(np.float32)
    valid = (yt >= 0).astype(np.float32)
    idx = np.clip(yt, 0, L - 1)

    PG = np.zeros((KT, B), np.float32)
    QG = np.zeros((KT, B), np.float32)
    PG[0:D_USE] = zn
    QG[0:D_USE] = 0.5 * zn
    PG[192] = 1.0
    QG[192] = 0.5
    PG[193] = bg
    QG[193] = -0.01 - 0.18 * bg
    PG[194] = 1.0
    QG[194] = -0.01 * bg
    oh = np.zeros((L, B), np.float32)
    oh[idx, np.arange(B)] = valid
    PG[195 : 195 + L] = oh
    QG[195 : 195 + L] = -lk[:, idx] * valid[None, :]
    return PG, QG


def _host_inputs(y_true, y_pred, lookup):
    """Build the 8 per-core input maps."""
    PG, QG = _pack_pq(y_true, y_pred, lookup)

    in_maps = []
    for r in range(N_CORES):
        qcore = np.zeros((KT, NCOL), np.float32)
        for d in range(NSLOT):
            if d == 4 and r >= 4:
                continue  # padded slot stays zero
            cb = (r + d) % NB
            w = 1.0 if d == 0 else 2.0
            qcore[:, d * BLK : (d + 1) * BLK] = (
                w * QG[:, cb * BLK : (cb + 1) * BLK]
            )
        pcore = PG[:, r * BLK : (r + 1) * BLK]
        p8 = pcore.astype(FP8)
        q8 = qcore.astype(FP8)
        # device layout: i128 = [pab (p,c,m) | qab (p,g,c,n)]
        #                i67  = [pc  (p,m)   | qc  (p,g,n)]
        pab = p8[0:256].reshape(2, 128, BLK).transpose(1, 0, 2)
        qab = (
            q8[0:256]
            .reshape(2, 128, NSLOT, BLK)
            .transpose(1, 2, 0, 3)
        )
        i128 = np.concatenate(
            [pab.reshape(128, 2 * BLK), qab.reshape(128, NSLOT * 1024)], axis=1
        )
        i67 = np.concatenate([p8[256:KT], q8[256:KT]], axis=1)
        in_maps.append(
            {
                "i128": np.ascontiguousarray(i128),
                "i67": np.ascontiguousarray(i67),
            }
        )
    return in_maps


def _combine(outs, y_true, lookup):
    """outs: list of 8 dicts with 'acc' [128, NUNIT]."""
    yt = np.asarray(y_true).astype(np.int64)
    lk = np.asarray(lookup).astype(np.float64)

    total = 0.0
    for r in range(N_CORES):
        total += float(outs[r]["acc"].astype(np.float64).sum())

    # diagonal correction: latent_ii = 1, target_ii = 0.2 (bg) or lookup[y,y]
    bgm = yt == -1
    idx = np.clip(yt, 0, L - 1)
    tdiag = np.where(bgm, 0.2, lk[idx, idx])
    diag_sum = float(np.abs(1.0 - tdiag).sum())

    n_pairs = B * (B - 1) // 2
    return np.float32((total - diag_sum) / 2.0 / n_pairs)


def kernel(y_true, y_pred, lookup):
    from concourse.bass_utils import run_bass_kernel_spmd

    nc = _get_nc()
    in_maps = _host_inputs(y_true, y_pred, lookup)
    res = run_bass_kernel_spmd(nc, in_maps, core_ids=list(range(N_CORES)))
    return _combine(res.results, y_true, lookup)


# revision 18
# speedup vs baseline: 1.0483x; 1.0483x over previous
"""AffinityCosineLoss on 8 Trainium2 NeuronCores — fp8 streaming matmul.

Math: with zn = l2norm(y_pred[:, :192]), latent = (zn@zn.T + 1)/2,
target[i,j] = 0.2 (both bg) / 0.01 (one bg) / lookup[y_i,y_j] (both valid),
loss = sum_{i<j} |latent - target| / (B*(B-1)/2).

The entire pairwise map latent - target is a single K=323 contraction
P.T @ Q, fully packed on the HOST (fp32 math, then fp8 cast):
  rows   0:192  P = zn_i.T            Q = 0.5 * zn_j.T
  row    192    P = 1                 Q = 0.5            (the +1/2 of latent)
  row    193    P = b_i               Q = -0.01 - 0.18*b_j
  row    194    P = 1                 Q = -0.01*b_j      (b = is_background)
  rows 195:323  P = onehot(y_i)       Q = -lookup[:, y_j] * valid_j
The asymmetric 1.0 x 0.5 const split keeps the fp8 constants exact.
K chunks: A = rows 0:128, B = rows 128:256, C = rows 256:323 (67).
Plain matmuls (no DoubleRow): full 128-col stationaries trigger the
compiler's Fast Weight Load, and --enable-ldw-opt dedupes the repeated
stationary across the slot-pair inner loop.

Sharding (triangle/cyclic): the 4096x4096 pair matrix is an 8x8 grid of
512x512 super-blocks. Core r computes blocks (r, (r+d) mod 8) for d=0..4;
the d=4 slot is zero-padded on cores 4..7. The x2 weight of off-diagonal
slots is baked into the Q columns (|2x| = 2|x|), so the device just
abs-sums everything. Host: total = sum - diag_correction, /2, /npairs.

Device: 20 out-tiles [128,512] in 3 waves of slots {0,1},{2,3},{4};
PSUM units [128,2,512] rotate through all 8 banks (tag bufs=4, first
rotation slot doubles as the PE-warmup target). Drains (abs + sum into
one acc column) alternate ACT (activation Abs accum_out) / DVE
(tensor_reduce) and overlap the next wave's matmuls.
"""

import functools

import ml_dtypes
import numpy as np

B = 4096
D = 256
L = 128
D_USE = 192  # int(D * 0.75)
NB = 8  # super-block grid (512 rows each)
BLK = B // NB  # 512
NSLOT = 5  # col slots per core (d = 0..4)
NCOL = NSLOT * BLK  # 2560
N_CORES = 8
NORM_EPS = 1e-8

KT = 323  # contraction rows
KC = 67  # rows of chunk C
NUNIT = 2 * NSLOT  # drain units: (slot, strip-half)

FP8 = ml_dtypes.float8_e4m3


def _enable_ldw_opt():
    """Flip walrus --enable-ldw-opt to true (dedupes back-to-back LDWEIGHTS
    with identical stationary operands; the main loop is ordered for it)."""
    import concourse.bass_utils as bu

    if getattr(bu, "_ldw_opt_patched", False):
        return
    orig = bu.run_command

    def run_command_ldw(argv, **kwargs):
        argv = [
            a.replace("--enable-ldw-opt=false", "--enable-ldw-opt=true")
            if isinstance(a, str)
            else a
            for a in argv
        ]
        return orig(argv, **kwargs)

    bu.run_command = run_command_ldw
    bu._ldw_opt_patched = True


def _build_bass():
    import concourse.bacc as bacc
    import concourse.mybir as mybir
    import concourse.tile as tile

    # NOTE: walrus --enable-ldw-opt rejects these fp8 InstLdweights
    # ("not compatible with LDW optimization"), so it stays off; the
    # PE's 64-deep reorder window still pulls LDWEIGHTS ahead.

    fp32 = mybir.dt.float32
    bf16 = mybir.dt.bfloat16
    f8 = mybir.dt.float8e4

    nc = bacc.Bacc("TRN2", debug=False, num_devices=N_CORES)

    # all 128-partition input in one tensor, all 67-partition input in the
    # other: 2 + 2 wave-sliced DMA issues move everything
    i128_d = nc.dram_tensor(
        "i128", [128, 1024 + NSLOT * 1024], f8, kind="ExternalInput"
    )
    i67_d = nc.dram_tensor("i67", [KC, BLK + NSLOT * BLK], f8, kind="ExternalInput")
    acc_d = nc.dram_tensor("acc", [128, NUNIT], fp32, kind="ExternalOutput")

    AX = mybir.AxisListType
    ALU = mybir.AluOpType
    ACTF = mybir.ActivationFunctionType

    with tile.TileContext(nc) as tc:
        with (
            tc.tile_pool(name="cst", bufs=1) as cst,
            tc.tile_pool(name="work", bufs=1) as work,
            tc.tile_pool(name="ps", bufs=1, space="PSUM") as pps,
        ):
            # ---- SBUF tiles: one tile per DMA so dependency tracking and
            # write-hazard windows are exact (a matmul only waits on — and
            # only aliases — its own wave's buffer) ----
            pabs = work.tile([128, 2, BLK], f8)
            pcs = work.tile([KC, BLK], f8)
            qabw = [
                work.tile([128, 2 if w < 2 else 1, 2, BLK], f8, name=f"qab{w}")
                for w in range(3)
            ]
            qcw = [
                work.tile([KC, 2 if w < 2 else 1, BLK], f8, name=f"qc{w}")
                for w in range(3)
            ]
            acc = work.tile([128, NUNIT], fp32)

            # ---- engine warmup + input DMAs ----
            # All input rides the fast sync HWDGE ring, need-ordered.
            # Scalar keeps only the warmup activation (Abs table) + drains.
            wz = cst.tile([128, 512], f8)
            nc.gpsimd.memset(wz[:], 0.0)
            wact = cst.tile([128, 1], fp32)
            nc.gpsimd.memset(wact[:], 1.0)

            def dma_ab(w):
                ab = slice(1024 + w * 2048, 1024 + min((w + 1) * 2048, 5120))
                nc.sync.dma_start(qabw[w][:], i128_d.ap()[:, ab])

            def dma_c(w):
                c = slice(BLK + w * 1024, BLK + min((w + 1) * 1024, NSLOT * BLK))
                nc.sync.dma_start(qcw[w][:], i67_d.ap()[:, c])

            nc.sync.dma_start(pabs[:], i128_d.ap()[:, 0:1024])
            nc.sync.dma_start(pcs[:], i67_d.ap()[:, 0:BLK])
            for w in range(3):
                dma_ab(w)
                dma_c(w)

            wabs = cst.tile([128, 1], fp32)
            nc.scalar.activation(wabs[:], wact[:], ACTF.Abs)

            wp = pps.tile([128, 2, BLK], fp32, tag="mm", bufs=4, name="wp")
            for wi in range(6):
                nc.tensor.matmul(
                    wp[:, wi % 2, :], wz[:, 0:128], wz[:], start=True, stop=True
                )

            # ---- main: 3 waves of slots {0,1}, {2,3}, {4} ----
            pending = []

            def drain(unit, u):
                if u % 2 == 0:
                    scr = work.tile([128, 2, BLK], bf16, tag="scr", bufs=2)
                    nc.scalar.activation(
                        scr[:], unit[:], ACTF.Abs, accum_out=acc[:, u : u + 1]
                    )
                else:
                    nc.vector.tensor_reduce(
                        acc[:, u : u + 1],
                        unit[:],
                        axis=AX.XY,
                        op=ALU.add,
                        apply_absolute_value=True,
                    )

            for wave in ((0, 1), (2, 3), (4,)):
                units = {}
                for g in wave:
                    for h in range(2):
                        units[(g, h)] = pps.tile(
                            [128, 2, BLK], fp32, tag="mm", bufs=4, name=f"u{g}_{h}"
                        )
                # drains of the previous wave overlap this wave's matmuls
                for unit, u in pending:
                    drain(unit, u)
                pending.clear()
                for m in range(4):
                    ms = slice(m * 128, (m + 1) * 128)
                    for c in range(3):
                        for g in wave:
                            w, gi = g // 2, g % 2
                            lhsT = pcs[:, ms] if c == 2 else pabs[:, c, ms]
                            rhs = (
                                qcw[w][:, gi, :]
                                if c == 2
                                else qabw[w][:, gi, c, :]
                            )
                            nc.tensor.matmul(
                                units[(g, m // 2)][:, m % 2, :],
                                lhsT,
                                rhs,
                                start=(c == 0),
                                stop=(c == 2),
                            )
                for g in wave:
                    for h in range(2):
                        pending.append((units[(g, h)], g * 2 + h))

            # ship the first 8 acc columns while wave 2 drains
            nc.sync.dma_start(acc_d.ap()[:, 0:8], acc[:, 0:8])
            for unit, u in pending:
                drain(unit, u)
            nc.sync.dma_start(acc_d.ap()[:, 8:NUNIT], acc[:, 8:NUNIT])

    nc.compile()
    return nc


@functools.lru_cache(maxsize=1)
def _get_nc():
    return _build_bass()


def _pack_pq(y_true, y_pred, lookup):
    """Global [KT, B] P and Q fp32 matrices (see module docstring)."""
    yt = np.asarray(y_true).astype(np.int64)
    yp = np.asarray(y_pred).astype(np.float32)[:, :D_USE]
    lk = np.asarray(lookup).astype(np.float32)

    n = np.maximum(np.sqrt((yp * yp).sum(axis=1, keepdims=True)), NORM_EPS)
    zn = (yp / n).T  # [192, B]
    bg = (yt == -1).astype(np.float32)
    valid = (yt >= 0).astype(np.float32)
    idx = np.clip(yt, 0, L - 1)

    PG = np.zeros((KT, B), np.float32)
    QG = np.zeros((KT, B), np.float32)
    PG[0:D_USE] = zn
    QG[0:D_USE] = 0.5 * zn
    PG[192] = 1.0
    QG[192] = 0.5
    PG[193] = bg
    QG[193] = -0.01 - 0.18 * bg
    PG[194] = 1.0
    QG[194] = -0.01 * bg
    oh = np.zeros((L, B), np.float32)
    oh[idx, np.arange(B)] = valid
    PG[195 : 195 + L] = oh
    QG[195 : 195 + L] = -lk[:, idx] * valid[None, :]
    return PG, QG


def _host_inputs(y_true, y_pred, lookup):
    """Build the 8 per-core input maps."""
    PG, QG = _pack_pq(y_true, y_pred, lookup)

    in_maps = []
    for r in range(N_CORES):
        qcore = np.zeros((KT, NCOL), np.float32)
        for d in range(NSLOT):
            if d == 4 and r >= 4:
                continue  # padded slot stays zero
            cb = (r + d) % NB
            w = 1.0 if d == 0 else 2.0
            qcore[:, d * BLK : (d + 1) * BLK] = (
                w * QG[:, cb * BLK : (cb + 1) * BLK]
            )
        pcore = PG[:, r * BLK : (r + 1) * BLK]
        p8 = pcore.astype(FP8)
        q8 = qcore.astype(FP8)
        # device layout: i128 = [pab (p,c,m) | qab (p,g,c,n)]
        #                i67  = [pc  (p,m)   | qc  (p,g,n)]
        pab = p8[0:256].reshape(2, 128, BLK).transpose(1, 0, 2)
        qab = (
            q8[0:256]
            .reshape(2, 128, NSLOT, BLK)
            .transpose(1, 2, 0, 3)
        )
        i128 = np.concatenate(
            [pab.reshape(128, 2 * BLK), qab.reshape(128, NSLOT * 1024)], axis=1
        )
        i67 = np.concatenate([p8[256:KT], q8[256:KT]], axis=1)
        in_maps.append(
            {
                "i128": np.ascontiguousarray(i128),
                "i67": np.ascontiguousarray(i67),
            }
        )
    return in_maps


def _combine(outs, y_true, lookup):
    """outs: list of 8 dicts with 'acc' [128, NUNIT]."""
    yt = np.asarray(y_true).astype(np.int64)
    lk = np.asarray(lookup).astype(np.float64)

    total = 0.0
    for r in range(N_CORES):
        total += float(outs[r]["acc"].astype(np.float64).sum())

    # diagonal correction: latent_ii = 1, target_ii = 0.2 (bg) or lookup[y,y]
    bgm = yt == -1
    idx = np.clip(yt, 0, L - 1)
    tdiag = np.where(bgm, 0.2, lk[idx, idx])
    diag_sum = float(np.abs(1.0 - tdiag).sum())

    n_pairs = B * (B - 1) // 2
    return np.float32((total - diag_sum) / 2.0 / n_pairs)


def kernel(y_true, y_pred, lookup):
    from concourse.bass_utils import run_bass_kernel_spmd

    nc = _get_nc()
    in_maps = _host_inputs(y_true, y_pred, lookup)
    res = run_bass_kernel_spmd(nc, in_maps, core_ids=list(range(N_CORES)))
    return _combine(res.results, y_true, lookup)
